# revision 1
# baseline (speedup 1.0000x reference)
"""Trainium2 Bass kernel for the KGEncoder RGCN (nn_KGEncoder_14027363188782).

Math (per batch element b, L=5 layers):
    x0 = ent_emb                                             (E, D)
    per layer i:
      y_r   = x @ Wb_x[i,r] + 1 * c[i,r]^T    (E, NB)  where c[i,r] = rel_r @ Wb_rel[i,r]
      Z     = sum_r adj_r @ y_r               (E, NB)  == sup @ Wb[i]  (deg term folded via c)
      h     = relu(Z @ Ww[i] + bias[i])
      g     = sigmoid(h @ Wh[i] + bh[i])
      x     = x + g * (h - x)
    out_b = sum_e x[e] * m[e] / max(sum_e m[e], 1)

Sharding: core c handles b = c // 2 (pair-replicated, no collectives).
adj is shipped pre-transposed (j-major) in bf16 (exact for 0/1 values).
Big matmul: out Z.T (NB x E) = sum_{r,k} y'[kchunk]_r.T @ adjT_r[kchunk];
NRES relations stay resident in SBUF, the rest stream from HBM each layer.
"""

import numpy as np
import ml_dtypes

import concourse.bacc as bacc
import concourse.bass as bass
import concourse.mybir as mybir
import concourse.tile as tile
from concourse import bass_utils
from concourse.bass import MemorySpace

B, R, E, D, HID, L, NB = 4, 10, 1500, 100, 100, 5, 3
EP = 1536           # entity (j) dim padded to 12*128
CH = EP // 128      # 12 k-chunks
FP8 = True          # fp8 adj (exact for 0/1) -> all relations SBUF-resident
DR = True           # DoubleRow fp8 matmul: 256-deep contraction, 2 elem/lane/cyc
C2 = 6              # 256-row contraction chunks (DoubleRow)
E2 = 1504           # i dim padded to 16-aligned for DoubleRow strides
YQ = 32             # y_all per-chunk col stride (16-aligned)
NRES = 10 if FP8 else 4   # relations resident in SBUF
SG = 3              # k-chunks per streamed stage tile
NW = 500            # psum free-dim chunk (3 per row of E)
RNB = R * NB        # 30
f32 = mybir.dt.float32
bf16 = mybir.dt.bfloat16
ADT = mybir.dt.float8e4 if FP8 else mybir.dt.bfloat16
ADT_NP = ml_dtypes.float8_e4m3fn if FP8 else ml_dtypes.bfloat16
AF = mybir.ActivationFunctionType
AX = mybir.AxisListType

_NC_CACHE = {}


def _build_nc():
    nc = bacc.Bacc("TRN2", target_bir_lowering=False, debug=False)

    if DR:
        adjT = nc.dram_tensor(
            "adjT", [R, C2, 128, 2, E2], ADT, kind="ExternalInput"
        ).ap()
    else:
        adjT = nc.dram_tensor("adjT", [R, EP, E], ADT, kind="ExternalInput").ap()
    xT0 = nc.dram_tensor("xT0", [D, E], f32, kind="ExternalInput").ap()
    maskrep = nc.dram_tensor("maskrep", [HID, E], f32, kind="ExternalInput").ap()
    relT = nc.dram_tensor("relT", [D, R], f32, kind="ExternalInput").ap()
    wbxD = nc.dram_tensor("wbx", [L, D, RNB], f32, kind="ExternalInput").ap()
    wbrD = nc.dram_tensor("wbr", [L, D, RNB], f32, kind="ExternalInput").ap()
    wwD = nc.dram_tensor("ww", [L, NB, HID], f32, kind="ExternalInput").ap()
    whD = nc.dram_tensor("wh", [L, HID, HID], f32, kind="ExternalInput").ap()
    biasD = nc.dram_tensor("biasL", [L, HID], f32, kind="ExternalInput").ap()
    bhD = nc.dram_tensor("bhL", [L, HID], f32, kind="ExternalInput").ap()
    graphD = nc.dram_tensor("graph", [HID, 1], f32, kind="ExternalOutput").ap()

    with tile.TileContext(nc) as tc:
        with (
            tc.tile_pool(name="singles", bufs=1) as singles,
            tc.tile_pool(name="resp", bufs=1) as resp,
            tc.tile_pool(name="stagep", bufs=4) as stagep,
            tc.tile_pool(name="ypool", bufs=2) as ypool,
            tc.tile_pool(name="workp", bufs=2) as workp,
            tc.tile_pool(name="psY", bufs=1, space=MemorySpace.PSUM) as psY,
            tc.tile_pool(name="psC", bufs=1, space=MemorySpace.PSUM) as psC,
            tc.tile_pool(name="psB", bufs=1, space=MemorySpace.PSUM) as psB,
        ):
            # ---- persistent state ----
            xT = singles.tile([D, EP], f32, tag="xT", name="xT")
            nc.sync.dma_start(out=xT[:, 0:E], in_=xT0)
            nc.vector.memset(xT[:, E:EP], 0.0)

            ones = singles.tile([1, 128], f32, tag="ones", name="ones")
            nc.vector.memset(ones[:, :], 1.0)

            mask_sb = singles.tile([HID, E], f32, tag="mask", name="mask_sb")
            nc.sync.dma_start(out=mask_sb[:, :], in_=maskrep)

            relT_sb = singles.tile([D, R], f32, tag="relT", name="relT_sb")
            nc.sync.dma_start(out=relT_sb[:, :], in_=relT)

            wbx_sb, wbr_sb, ww_sb, wh_sb, bias_sb, bh_sb = [], [], [], [], [], []
            for i in range(L):
                wx = singles.tile([D, RNB], f32, tag=f"wbx{i}", name=f"wbx{i}")
                nc.sync.dma_start(out=wx[:, :], in_=wbxD[i])
                wbx_sb.append(wx)
                wr = singles.tile([D, RNB], f32, tag=f"wbr{i}", name=f"wbr{i}")
                nc.sync.dma_start(out=wr[:, :], in_=wbrD[i])
                wbr_sb.append(wr)
                wwt = singles.tile([NB, HID], f32, tag=f"ww{i}", name=f"ww{i}")
                nc.sync.dma_start(out=wwt[:, :], in_=wwD[i])
                ww_sb.append(wwt)
                wht = singles.tile([HID, HID], f32, tag=f"wh{i}", name=f"wh{i}")
                nc.sync.dma_start(out=wht[:, :], in_=whD[i])
                wh_sb.append(wht)
                bt = singles.tile([HID, 1], f32, tag=f"bias{i}", name=f"bias{i}")
                nc.sync.dma_start(out=bt[:, :], in_=biasD[i].unsqueeze(1))
                bias_sb.append(bt)
                bht = singles.tile([HID, 1], f32, tag=f"bh{i}", name=f"bh{i}")
                nc.sync.dma_start(out=bht[:, :], in_=bhD[i].unsqueeze(1))
                bh_sb.append(bht)

            # resident adjT relations: tile (128, CH*E), chunk k at cols [k*E, (k+1)*E)
            res_tiles = []
            for r in range(NRES):
                if DR:
                    rt = resp.tile([128, C2 * 2 * E2], ADT,
                                   tag=f"res{r}", name=f"res{r}")
                    nc.sync.dma_start(
                        out=rt[:, :].rearrange("p (c t i) -> p c t i", c=C2, t=2),
                        in_=adjT[r].rearrange("c p t i -> p c t i"),
                    )
                else:
                    rt = resp.tile([128, CH * E], ADT, tag=f"res{r}", name=f"res{r}")
                    nc.sync.dma_start(
                        out=rt[:, :].rearrange("p (k i) -> p k i", k=CH),
                        in_=adjT[r].rearrange("(k p) i -> p k i", p=128),
                    )
                res_tiles.append(rt)

            # ---- layers ----
            for i in range(L):
                # c[i, r, :] = rel_r @ Wb_rel[i, r]   -> psum row 0, cols 3r..3r+3
                psc = psC.tile([1, RNB], f32, tag="c", name=f"psc{i}")
                for r in range(R):
                    nc.tensor.matmul(
                        psc[:, 3 * r : 3 * r + 3],
                        relT_sb[:, r : r + 1],
                        wbr_sb[i][:, 3 * r : 3 * r + 3],
                        start=True, stop=True,
                    )
                c_sb = workp.tile([1, RNB], f32, tag="c_sb", name=f"c_sb{i}", bufs=2)
                nc.scalar.copy(out=c_sb[:, :], in_=psc[:, :])

                # y'[kchunk] = x[kchunk] @ Wbx[i]  + 1 (x) c   -> bf16 (128, RNB) per chunk
                YS = YQ if DR else RNB
                y_all = ypool.tile([128, CH * YS], ADT, tag="y_all", name=f"y_all{i}")
                for k in range(CH):
                    psy = psY.tile([128, RNB], f32, tag="y", name=f"psy{i}_{k}")
                    nc.tensor.matmul(
                        psy[:, :], xT[:, k * 128 : (k + 1) * 128], wbx_sb[i][:, :],
                        start=True, stop=False,
                    )
                    nc.tensor.matmul(
                        psy[:, :], ones[:, :], c_sb[:, :],
                        start=False, stop=True,
                    )
                    nc.scalar.copy(out=y_all[:, k * YS : k * YS + RNB], in_=psy[:, :])

                # Z.T (NB, E) = sum_{r, k} y'_r[k].T @ adjT_r[k]
                # per i-chunk n: accumulate Z chunk, then basis/highway tail on
                # ACT/DVE overlaps the next chunk's PE matmuls
                assert DR
                h_sb = workp.tile([HID, E], f32, tag="h", name=f"h_sb{i}", bufs=1)
                y_view = y_all[:, :].rearrange("p (k q) -> p k q", q=YQ)
                res_views = [
                    res_tiles[r][:, :].rearrange("p (c t i) -> p c t i", c=C2, t=2)
                    for r in range(R)
                ]
                for n in range(3):
                    ns = slice(n * NW, (n + 1) * NW)
                    psz = psB.tile([NB, 512], f32, tag="zz", bufs=2,
                                   name=f"psz{i}_{n}")
                    cnt = 0
                    for r in range(R):
                        for c in range(C2):
                            nc.tensor.matmul(
                                psz[:, 0:NW],
                                y_view[:, 2 * c : 2 * c + 2, 3 * r : 3 * r + 3],
                                res_views[r][:, c, :, ns],
                                start=(cnt == 0),
                                stop=(cnt == R * C2 - 1),
                                perf_mode=mybir.MatmulPerfMode.DoubleRow,
                            )
                            cnt += 1
                    z_sb = workp.tile([NB, NW], f32, tag="z_sb", bufs=2,
                                      name=f"z_sb{i}_{n}")
                    nc.scalar.copy(out=z_sb[:, :], in_=psz[:, 0:NW])
                    psh = psB.tile([HID, 512], f32, tag="hh", bufs=1,
                                   name=f"psh{i}_{n}")
                    nc.tensor.matmul(
                        psh[:, 0:NW], ww_sb[i][:, :], z_sb[:, :],
                        start=True, stop=True,
                    )
                    nc.scalar.activation(
                        h_sb[:, ns], psh[:, 0:NW], AF.Relu, bias=bias_sb[i][:, :],
                    )
                    psg = psB.tile([HID, 512], f32, tag="gg", bufs=1,
                                   name=f"psg{i}_{n}")
                    nc.tensor.matmul(
                        psg[:, 0:NW], wh_sb[i][:, :], h_sb[:, ns],
                        start=True, stop=True,
                    )
                    nc.scalar.activation(
                        psg[:, 0:NW], psg[:, 0:NW], AF.Sigmoid, bias=bh_sb[i][:, :],
                    )
                    # x = x + g * (h - x)  (chunk n)
                    nc.vector.tensor_sub(h_sb[:, ns], h_sb[:, ns], xT[:, ns])
                    nc.vector.tensor_mul(h_sb[:, ns], h_sb[:, ns], psg[:, 0:NW])
                    nc.vector.tensor_add(xT[:, ns], xT[:, ns], h_sb[:, ns])

            # ---- masked mean over entities ----
            xm = workp.tile([HID, E], f32, tag="h", name="xm", bufs=1)
            nc.vector.tensor_mul(xm[:, :], xT[:, 0:E], mask_sb[:, :])
            gsum = workp.tile([HID, 1], f32, tag="gsum", name="gsum", bufs=1)
            nc.vector.reduce_sum(gsum[:, :], xm[:, :], axis=AX.X)
            den = workp.tile([HID, 1], f32, tag="den", name="den", bufs=1)
            nc.vector.reduce_sum(den[:, :], mask_sb[:, :], axis=AX.X)
            nc.vector.tensor_scalar_max(den[:, :], den[:, :], 1.0)
            nc.vector.reciprocal(den[:, :], den[:, :])
            nc.vector.tensor_mul(gsum[:, :], gsum[:, :], den[:, :])
            nc.sync.dma_start(out=graphD, in_=gsum[:, :])

    nc.compile()
    return nc


def get_nc():
    if "nc" not in _NC_CACHE:
        _NC_CACHE["nc"] = _build_nc()
    return _NC_CACHE["nc"]


def make_in_maps(adj, mask_ids, ent_emb, rel_emb, Wb, Ww, bias, Wh, bh):
    adj = np.asarray(adj, dtype=np.float32)
    if DR:
        pad = np.zeros((B, R, EP, E2), dtype=ADT_NP)
        pad[:, :, :E, :E] = adj.transpose(0, 1, 3, 2).astype(ADT_NP)
        # [b, r, c, p, t, i] = adj[b, r, i, j = c*256 + t*128 + p]
        adjT = np.ascontiguousarray(
            pad.reshape(B, R, C2, 2, 128, E2).transpose(0, 1, 2, 4, 3, 5)
        )
    else:
        adjT = np.zeros((B, R, EP, E), dtype=ADT_NP)
        adjT[:, :, :E, :] = adj.transpose(0, 1, 3, 2).astype(ADT_NP)
    entT = np.ascontiguousarray(np.asarray(ent_emb, np.float32).T)
    relTh = np.ascontiguousarray(np.asarray(rel_emb, np.float32).T)
    Wb5 = np.asarray(Wb, np.float32).reshape(L, R, 2, D, NB)
    wbx = np.ascontiguousarray(Wb5[:, :, 0].transpose(0, 2, 1, 3).reshape(L, D, RNB))
    wbr = np.ascontiguousarray(Wb5[:, :, 1].transpose(0, 2, 1, 3).reshape(L, D, RNB))
    maskf = np.asarray(mask_ids).astype(np.float32)
    common = dict(
        xT0=entT, relT=relTh, wbx=wbx, wbr=wbr,
        ww=np.ascontiguousarray(np.asarray(Ww, np.float32)),
        wh=np.ascontiguousarray(np.asarray(Wh, np.float32)),
        biasL=np.ascontiguousarray(np.asarray(bias, np.float32)),
        bhL=np.ascontiguousarray(np.asarray(bh, np.float32)),
    )
    in_maps = []
    for c in range(8):
        b = c // 2
        m = dict(common)
        m["adjT"] = np.ascontiguousarray(adjT[b])
        m["maskrep"] = np.ascontiguousarray(
            np.broadcast_to(maskf[b][None, :], (HID, E))
        )
        in_maps.append(m)
    return in_maps


def run(inputs, trace=False):
    nc = get_nc()
    in_maps = make_in_maps(**{k: np.asarray(v) for k, v in inputs.items()})
    res = bass_utils.run_bass_kernel_spmd(
        nc, in_maps, core_ids=list(range(8)), trace=trace
    )
    out = np.stack(
        [np.asarray(res.results[2 * b]["graph"]).reshape(HID) for b in range(B)]
    ).astype(np.float32)
    return out, res


def kernel(**inputs):
    out, _ = run(inputs, trace=False)
    return out



# revision 16
# speedup vs baseline: 1.4960x; 1.4960x over previous
"""Trainium2 Bass kernel for the KGEncoder RGCN (nn_KGEncoder_14027363188782).

Math (per batch element b, L=5 layers):
    x0 = ent_emb                                             (E, D)
    per layer i:
      y_r   = x @ Wb_x[i,r] + 1 * c[i,r]^T    (E, NB)  where c[i,r] = rel_r @ Wb_rel[i,r]
      Z     = sum_r adj_r @ y_r               (E, NB)
      h     = relu(Z @ Ww[i] + bias[i])
      g     = sigmoid(h @ Wh[i] + bh[i])
      x     = x + g * (h - x)
    out_b = sum_e x[e] * m[e] / max(sum_e m[e], 1)

Sharding: core c handles b = c // 2 (pair-replicated, no collectives).

Big matmul orientation: stationary = adjT 256x128 DoubleRow blocks
(fp8, exact for 0/1), moving = y chunk (256, 3) -> psum (128 i, 3).
Z chunks are PE-transposed back to (3, E) for the basis/highway tail.
"""

import numpy as np
import ml_dtypes

import concourse.bacc as bacc
import concourse.bass as bass
import concourse.mybir as mybir
import concourse.tile as tile
from concourse import bass_utils
from concourse.bass import MemorySpace

B, R, E, D, HID, L, NB = 4, 10, 1500, 100, 100, 5, 3
EP = 1536           # entity (j) dim padded to 12*128
CH = EP // 128      # 12 j-chunks of 128
C2 = 6              # 256-row contraction chunks (DoubleRow)
E2 = 1504           # i dim padded (16-aligned, 11.75 chunks -> last is 96)
IC = 12             # i chunks of 128 (last covers 1504-1408=96... see ISLICE)
YQ = 32             # y_all per-chunk col stride (16-aligned)
RNB = R * NB        # 30
f32 = mybir.dt.float32
bf16 = mybir.dt.bfloat16
fp8 = mybir.dt.float8e4
AF = mybir.ActivationFunctionType
AX = mybir.AxisListType
DR = mybir.MatmulPerfMode.DoubleRow
f32r = mybir.dt.float32r

# i chunking: 12 chunks; chunks 0..10 are 128 wide, chunk 11 is 96 (1504 total)
ISL = [(k * 128, min(128, E2 - k * 128)) for k in range(IC)]
# free-dim chunking of E2 for the tail (psum bank = 512 f32)
NSL = [(0, 512), (512, 512), (1024, 480)]

_NC_CACHE = {}


def _build_nc():
    nc = bacc.Bacc("TRN2", target_bir_lowering=False, debug=False)

    adjT = nc.dram_tensor("adjT", [R, C2, 128, 2, E2], fp8, kind="ExternalInput").ap()
    xT0 = nc.dram_tensor("xT0", [D, EP], f32, kind="ExternalInput").ap()
    maskrep = nc.dram_tensor("maskrep", [128, CH], f32, kind="ExternalInput").ap()
    relT = nc.dram_tensor("relT", [D, R], f32, kind="ExternalInput").ap()
    ident = nc.dram_tensor("ident", [128, 128], f32, kind="ExternalInput").ap()
    wbxD = nc.dram_tensor("wbx", [L, D, RNB], f32, kind="ExternalInput").ap()
    wbrD = nc.dram_tensor("wbr", [L, D, RNB], f32, kind="ExternalInput").ap()
    wwD = nc.dram_tensor("ww", [L, NB, HID], f32, kind="ExternalInput").ap()
    whD = nc.dram_tensor("wh", [L, HID, HID], f32, kind="ExternalInput").ap()
    biasD = nc.dram_tensor("biasL", [L, HID], f32, kind="ExternalInput").ap()
    bhD = nc.dram_tensor("bhL", [L, HID], f32, kind="ExternalInput").ap()
    graphD = nc.dram_tensor("graph", [HID, 1], f32, kind="ExternalOutput").ap()

    with tile.TileContext(nc) as tc:
        with (
            tc.tile_pool(name="singles", bufs=1) as singles,
            tc.tile_pool(name="resp", bufs=1) as resp,
            tc.tile_pool(name="ypool", bufs=2) as ypool,
            tc.tile_pool(name="workp", bufs=2) as workp,
            tc.tile_pool(name="psY", bufs=1, space=MemorySpace.PSUM) as psY,
            tc.tile_pool(name="psZ", bufs=1, space=MemorySpace.PSUM) as psZ,
            tc.tile_pool(name="psT", bufs=1, space=MemorySpace.PSUM) as psT,
            tc.tile_pool(name="psB", bufs=2, space=MemorySpace.PSUM) as psB,
        ):
            # ---- small persistent state (loaded before the big adj DMAs) ----
            xT = singles.tile([D, EP], f32, tag="xT", name="xT")
            nc.sync.dma_start(out=xT[:, :], in_=xT0)

            ident_sb = singles.tile([128, 128], f32, tag="ident", name="ident_sb")
            nc.sync.dma_start(out=ident_sb[:, :], in_=ident)

            ones = singles.tile([1, 128], f32, tag="ones", name="ones")
            nc.vector.memset(ones[:, :], 1.0)

            relT_sb = singles.tile([D, R], f32, tag="relT", name="relT_sb")
            nc.sync.dma_start(out=relT_sb[:, :], in_=relT)

            wbx_sb = singles.tile([D, L * RNB], f32, tag="wbx", name="wbx_sb")
            nc.sync.dma_start(
                out=wbx_sb[:, :].rearrange("p (l q) -> p l q", l=L),
                in_=wbxD.rearrange("l p q -> p l q"),
            )
            wbr_sb = singles.tile([D, L * RNB], f32, tag="wbr", name="wbr_sb")
            nc.sync.dma_start(
                out=wbr_sb[:, :].rearrange("p (l q) -> p l q", l=L),
                in_=wbrD.rearrange("l p q -> p l q"),
            )
            ww_sb = singles.tile([NB, L * HID], f32, tag="ww", name="ww_sb")
            nc.sync.dma_start(
                out=ww_sb[:, :].rearrange("p (l q) -> p l q", l=L),
                in_=wwD.rearrange("l p q -> p l q"),
            )
            wh_sb = singles.tile([HID, L * HID], f32, tag="wh", name="wh_sb")
            nc.sync.dma_start(
                out=wh_sb[:, :].rearrange("p (l q) -> p l q", l=L),
                in_=whD.rearrange("l p q -> p l q"),
            )
            bias_sb = singles.tile([HID, L], f32, tag="bias", name="bias_sb")
            nc.sync.dma_start(out=bias_sb[:, :], in_=biasD.rearrange("l p -> p l"))
            bh_sb = singles.tile([HID, L], f32, tag="bh", name="bh_sb")
            nc.sync.dma_start(out=bh_sb[:, :], in_=bhD.rearrange("l p -> p l"))

            mask_sb = singles.tile([128, CH], f32, tag="mask", name="mask_sb")
            nc.sync.dma_start(out=mask_sb[:, :], in_=maskrep)
            ones128 = singles.tile([128, 1], f32, tag="ones128", name="ones128")
            nc.vector.memset(ones128[:, :], 1.0)

            # ---- resident adjT relations: (p, (c t i)) fp8 ----
            res_tiles = []
            for r in range(R):
                rt = resp.tile([128, C2 * 2 * E2], fp8, tag=f"res{r}", name=f"res{r}")
                nc.sync.dma_start(
                    out=rt[:, :].rearrange("p (c t i) -> p c t i", c=C2, t=2),
                    in_=adjT[r].rearrange("c p t i -> p c t i"),
                )
                res_tiles.append(rt)
            res_views = [
                res_tiles[r][:, :].rearrange("p (c t i) -> p c t i", c=C2, t=2)
                for r in range(R)
            ]

            # ---- layers ----
            for i in range(L):
                wbx_i = wbx_sb[:, i * RNB : (i + 1) * RNB]
                wbr_i = wbr_sb[:, i * RNB : (i + 1) * RNB]
                ww_i = ww_sb[:, i * HID : (i + 1) * HID]
                wh_i = wh_sb[:, i * HID : (i + 1) * HID]
                bias_i = bias_sb[:, i : i + 1]
                bh_i = bh_sb[:, i : i + 1]

                # c[r, :] = rel_r @ Wb_rel[i, r] -> psy partition 0, tail cols
                psy = psY.tile([128, CH * RNB + YQ], f32, tag="y", name=f"psy{i}")
                for r in range(R):
                    nc.tensor.matmul(
                        psy[0:1, CH * RNB + 3 * r : CH * RNB + 3 * r + 3],
                        relT_sb[:, r : r + 1],
                        wbr_i[:, 3 * r : 3 * r + 3],
                        start=True, stop=True,
                    )
                c_sb = workp.tile([1, RNB], f32, tag="c_sb", name=f"c_sb{i}", bufs=2)
                nc.scalar.copy(out=c_sb[:, :], in_=psy[0:1, CH * RNB : CH * RNB + RNB])

                # y[kchunk] = x[kchunk] @ Wbx[i] + 1 (x) c  -> fp8 (128, 30)/chunk
                for k in range(CH):
                    ks = slice(k * RNB, (k + 1) * RNB)
                    nc.tensor.matmul(
                        psy[:, ks], xT[:, k * 128 : (k + 1) * 128], wbx_i,
                        start=True, stop=False,
                    )
                    nc.tensor.matmul(
                        psy[:, ks], ones[:, :], c_sb[:, :],
                        start=False, stop=True,
                    )
                y_all = ypool.tile([128, CH * YQ], fp8, tag="y_all", name=f"y_all{i}")
                nc.scalar.copy(
                    out=y_all[:, :].rearrange("p (k q) -> p k q", k=CH)[:, :, 0:RNB],
                    in_=psy[:, 0 : CH * RNB].rearrange("p (k q) -> p k q", k=CH),
                )
                y_view = y_all[:, :].rearrange("p (k q) -> p k q", q=YQ)

                # Z[ic] (128 i, 3) += adjT_block.T @ y_chunk   (DoubleRow fp8)
                zps = psZ.tile([128, IC * 8], f32, tag="z", name=f"zps{i}")
                for ic in range(IC):
                    i0, iw = ISL[ic]
                    for r in range(R):
                        for c in range(C2):
                            nc.tensor.matmul(
                                zps[0:iw, ic * 8 : ic * 8 + NB],
                                res_views[r][:, c, :, i0 : i0 + iw],
                                y_view[:, 2 * c : 2 * c + 2, 3 * r : 3 * r + 3],
                                start=(r == 0 and c == 0),
                                stop=(r == R - 1 and c == C2 - 1),
                                perf_mode=DR,
                            )
                zc_sb = workp.tile([128, IC * NB], f32, tag="zc",
                                   name=f"zc{i}", bufs=2)
                nc.scalar.copy(
                    out=zc_sb[:, :].rearrange("p (k w) -> p k w", w=NB),
                    in_=zps[:, :].rearrange("p (k w) -> p k w", w=8)[:, :, 0:NB])

                # transpose Z chunks -> zT (3, E2) bf16
                zt_ps = [
                    psT.tile([NB, 512], f32, tag="zt0", name=f"zt0_{i}", bufs=1),
                    psT.tile([NB, 512], f32, tag="zt1", name=f"zt1_{i}", bufs=1),
                    psT.tile([NB, 512], f32, tag="zt2", name=f"zt2_{i}", bufs=1),
                ]
                for ic in range(IC):
                    i0, iw = ISL[ic]
                    pst = zt_ps[ic // 4]
                    off = (ic % 4) * 128
                    nc.tensor.transpose(
                        pst[:, off : off + iw],
                        zc_sb[0:iw, ic * NB : (ic + 1) * NB],
                        ident_sb[0:iw, 0:iw],
                    )
                zT_chunks = []
                for n in range(3):
                    n0, nw = NSL[n]
                    ztc = workp.tile([NB, 512], f32, tag="zT", name=f"zT{i}_{n}",
                                     bufs=2)
                    nc.vector.tensor_copy(ztc[:, 0:nw], zt_ps[n][:, 0:nw])
                    zT_chunks.append(ztc)

                # tail: h = relu(Z @ Ww + bias); g = sigmoid(h @ Wh + bh);
                # x += g * (h - x)
                for n in range(3):
                    n0, nw = NSL[n]
                    ns = slice(n0, n0 + nw)
                    psh = psB.tile([HID, 512], f32, tag="hh", bufs=1,
                                   name=f"psh{i}_{n0}")
                    nc.tensor.matmul(
                        psh[:, 0:nw], ww_i, zT_chunks[n][:, 0:nw],
                        start=True, stop=True,
                    )
                    hc = workp.tile([HID, 512], f32, tag="h", name=f"h{i}_{n}",
                                    bufs=2)
                    nc.scalar.activation(
                        hc[:, 0:nw], psh[:, 0:nw], AF.Relu, bias=bias_i,
                    )
                    psg = psB.tile([HID, 512], f32, tag="gg", bufs=2,
                                   name=f"psg{i}_{n0}")
                    nc.tensor.matmul(
                        psg[:, 0:nw], wh_i, hc[:, 0:nw],
                        start=True, stop=True,
                    )
                    gc = workp.tile([HID, 512], f32, tag="g", name=f"g{i}_{n}",
                                    bufs=2)
                    nc.scalar.activation(
                        gc[:, 0:nw], psg[:, 0:nw], AF.Sigmoid, bias=bh_i,
                    )
                    nc.vector.tensor_sub(hc[:, 0:nw], hc[:, 0:nw], xT[:, ns])
                    nc.vector.tensor_mul(hc[:, 0:nw], hc[:, 0:nw], gc[:, 0:nw])
                    nc.vector.tensor_add(xT[:, ns], xT[:, ns], hc[:, 0:nw])

            # ---- masked mean over entities ----
            # gsum[d] = sum_k (xT chunk k).T[e, d] * maskP[e, k] via PE:
            # transpose each (100,128) x chunk to (128,100), then accumulate
            # matmul with the mask column as stationary.
            gsum_ps = psB.tile([HID, 512], f32, tag="gg", bufs=2, name="gsum_ps")
            for k in range(CH):
                xt_ps = psY.tile([128, CH * RNB + YQ], f32, tag="y",
                                 name=f"xtp{k}")
                nc.tensor.transpose(
                    xt_ps[:, 0:HID],
                    xT[:, k * 128 : (k + 1) * 128],
                    ident_sb[0:HID, 0:HID],
                )
                x_im = workp.tile([128, HID], f32, tag="x_im",
                                  name=f"x_im{k}", bufs=2)
                nc.scalar.copy(out=x_im[:, :], in_=xt_ps[:, 0:HID])
                nc.tensor.matmul(
                    gsum_ps[:, 0:1], x_im[:, :], mask_sb[:, k : k + 1],
                    start=(k == 0), stop=(k == CH - 1),
                )
            # den = max(sum(mask), 1); gsum /= den
            mrow = workp.tile([128, 1], f32, tag="mrow", name="mrow", bufs=1)
            nc.vector.reduce_sum(mrow[:, :], mask_sb[:, :], axis=AX.X)
            den_ps = psB.tile([HID, 512], f32, tag="hh", bufs=1, name="den_ps")
            nc.tensor.matmul(
                den_ps[0:1, 0:1], ones128[:, :], mrow[:, :], start=True, stop=True,
            )
            den1 = workp.tile([1, 1], f32, tag="den", name="den", bufs=1)
            nc.vector.tensor_scalar_max(den1[:, :], den_ps[0:1, 0:1], 1.0)
            nc.vector.reciprocal(den1[:, :], den1[:, :])
            psd = psB.tile([HID, 512], f32, tag="hh", bufs=1, name="psd")
            nc.tensor.matmul(
                psd[:, 0:1], ones[0:1, 0:HID], den1[:, :], start=True, stop=True,
            )
            rden = workp.tile([HID, 1], f32, tag="rden", name="rden", bufs=1)
            nc.scalar.copy(out=rden[:, :], in_=psd[:, 0:1])
            gsum = workp.tile([HID, 1], f32, tag="gsum", name="gsum", bufs=1)
            nc.vector.tensor_mul(gsum[:, :], gsum_ps[:, 0:1], rden[:, :])
            nc.sync.dma_start(out=graphD, in_=gsum[:, :])

    nc.compile()
    return nc


def get_nc():
    if "nc" not in _NC_CACHE:
        _NC_CACHE["nc"] = _build_nc()
    return _NC_CACHE["nc"]


def make_in_maps(adj, mask_ids, ent_emb, rel_emb, Wb, Ww, bias, Wh, bh):
    adj = np.asarray(adj, dtype=np.float32)
    pad = np.zeros((B, R, EP, E2), dtype=ml_dtypes.float8_e4m3fn)
    pad[:, :, :E, :E] = adj.transpose(0, 1, 3, 2).astype(ml_dtypes.float8_e4m3fn)
    # [b, r, c, p, t, i] = adj[b, r, i, j = c*256 + t*128 + p]
    adjT = np.ascontiguousarray(
        pad.reshape(B, R, C2, 2, 128, E2).transpose(0, 1, 2, 4, 3, 5)
    )
    entT = np.zeros((D, EP), dtype=np.float32)
    entT[:, :E] = np.asarray(ent_emb, np.float32).T
    relTh = np.ascontiguousarray(np.asarray(rel_emb, np.float32).T)
    Wb5 = np.asarray(Wb, np.float32).reshape(L, R, 2, D, NB)
    wbx = np.ascontiguousarray(
        Wb5[:, :, 0].transpose(0, 2, 1, 3).reshape(L, D, RNB)
    )
    wbr = np.ascontiguousarray(
        Wb5[:, :, 1].transpose(0, 2, 1, 3).reshape(L, D, RNB)
    )
    maskf = np.asarray(mask_ids).astype(np.float32)
    common = dict(
        xT0=entT, relT=relTh, wbx=wbx, wbr=wbr,
        ident=np.eye(128, dtype=np.float32),
        ww=np.ascontiguousarray(np.asarray(Ww, np.float32)),
        wh=np.ascontiguousarray(np.asarray(Wh, np.float32)),
        biasL=np.ascontiguousarray(np.asarray(bias, np.float32)),
        bhL=np.ascontiguousarray(np.asarray(bh, np.float32)),
    )
    in_maps = []
    for c in range(8):
        b = c // 2
        m = dict(common)
        m["adjT"] = np.ascontiguousarray(adjT[b])
        mp = np.zeros((EP,), dtype=np.float32)
        mp[:E] = maskf[b]
        m["maskrep"] = np.ascontiguousarray(mp.reshape(CH, 128).T)
        in_maps.append(m)
    return in_maps


def run(inputs, trace=False):
    nc = get_nc()
    in_maps = make_in_maps(**{k: np.asarray(v) for k, v in inputs.items()})
    res = bass_utils.run_bass_kernel_spmd(
        nc, in_maps, core_ids=list(range(8)), trace=trace
    )
    out = np.stack(
        [np.asarray(res.results[2 * b]["graph"]).reshape(HID) for b in range(B)]
    ).astype(np.float32)
    return out, res


def kernel(**inputs):
    out, _ = run(inputs, trace=False)
    return out


# revision 17
# speedup vs baseline: 1.7024x; 1.1380x over previous
"""Trainium2 Bass kernel for the KGEncoder RGCN (nn_KGEncoder_14027363188782).

Math (per batch element b, L=5 layers):
    x0 = ent_emb                                             (E, D)
    per layer i:
      y_r   = x @ Wb_x[i,r] + 1 * c[i,r]^T    (E, NB)  where c[i,r] = rel_r @ Wb_rel[i,r]
      Z     = sum_r adj_r @ y_r               (E, NB)
      h     = relu(Z @ Ww[i] + bias[i])
      g     = sigmoid(h @ Wh[i] + bh[i])
      x     = x + g * (h - x)
    out_b = sum_e x[e] * m[e] / max(sum_e m[e], 1)

Sharding: core c handles b = c // 2 (pair-replicated, no collectives).

Big matmul orientation: stationary = adjT 256x128 DoubleRow blocks
(fp8, exact for 0/1), moving = y chunk (256, 3) -> psum (128 i, 3).
Z chunks are PE-transposed back to (3, E) for the basis/highway tail.
"""

import numpy as np
import ml_dtypes

import concourse.bacc as bacc
import concourse.bass as bass
import concourse.mybir as mybir
import concourse.tile as tile
from concourse import bass_utils
from concourse.bass import MemorySpace

B, R, E, D, HID, L, NB = 4, 10, 1500, 100, 100, 5, 3
EP = 1536           # entity (j) dim padded to 12*128
CH = EP // 128      # 12 j-chunks of 128
C2 = 6              # 256-row contraction chunks (DoubleRow)
E2 = 1504           # i dim padded (16-aligned, 11.75 chunks -> last is 96)
IC = 12             # i chunks of 128 (last covers 1504-1408=96... see ISLICE)
YQ = 32             # y_all per-chunk col stride (16-aligned)
RNB = R * NB        # 30
f32 = mybir.dt.float32
bf16 = mybir.dt.bfloat16
fp8 = mybir.dt.float8e4
AF = mybir.ActivationFunctionType
AX = mybir.AxisListType
DR = mybir.MatmulPerfMode.DoubleRow
f32r = mybir.dt.float32r

# i chunking: 12 chunks; chunks 0..10 are 128 wide, chunk 11 is 96 (1504 total)
ISL = [(k * 128, min(128, E2 - k * 128)) for k in range(IC)]
# free-dim chunking of E2 for the tail (psum bank = 512 f32)
NSL = [(0, 512), (512, 512), (1024, 480)]

# f32 const blob layout (cols), partitions used in parens
OF_IDF = 0            # identity f32 (128)
OF_WBX = 128          # (100) L*RNB
OF_WBR = 278          # (100)
OF_BIAS = 428         # (100) L
OF_BH = 433           # (100) L
OF_RELT = 438         # (100) R
OF_MASK = 448         # (128) CH
OF_ONES = 460         # (1) 128 ones row
OF_ONES128 = 588      # (128) 1 ones column
CBF = 589
# bf16 const blob layout
OF_IDH = 0            # identity bf16 (128)
OF_WW = 128           # (3) L*HID
OF_WH = 628           # (100) L*HID
CBH = 1128

_NC_CACHE = {}


def _build_nc():
    nc = bacc.Bacc("TRN2", target_bir_lowering=False, debug=False)

    adjT = nc.dram_tensor("adjT", [R, C2, 128, 2, E2], fp8, kind="ExternalInput").ap()
    xT0 = nc.dram_tensor("xT0", [D, EP], f32, kind="ExternalInput").ap()
    cstFD = nc.dram_tensor("cstF", [128, CBF], f32, kind="ExternalInput").ap()
    cstHD = nc.dram_tensor("cstH", [128, CBH], bf16, kind="ExternalInput").ap()
    graphD = nc.dram_tensor("graph", [HID, 1], f32, kind="ExternalOutput").ap()

    with tile.TileContext(nc) as tc:
        with (
            tc.tile_pool(name="singles", bufs=1) as singles,
            tc.tile_pool(name="resp", bufs=1) as resp,
            tc.tile_pool(name="ypool", bufs=2) as ypool,
            tc.tile_pool(name="workp", bufs=2) as workp,
            tc.tile_pool(name="psY", bufs=1, space=MemorySpace.PSUM) as psY,
            tc.tile_pool(name="psZ", bufs=1, space=MemorySpace.PSUM) as psZ,
            tc.tile_pool(name="psT", bufs=1, space=MemorySpace.PSUM) as psT,
            tc.tile_pool(name="psB", bufs=2, space=MemorySpace.PSUM) as psB,
        ):
            # ---- small persistent state (loaded before the big adj DMAs) ----
            cstF = singles.tile([128, CBF], f32, tag="cstF", name="cstF")
            nc.sync.dma_start(out=cstF[:, :], in_=cstFD)
            cstH = singles.tile([128, CBH], bf16, tag="cstH", name="cstH")
            nc.sync.dma_start(out=cstH[:, :], in_=cstHD)
            xT = singles.tile([D, EP], f32, tag="xT", name="xT")
            nc.sync.dma_start(out=xT[:, :], in_=xT0)

            wbx_sb = cstF[0:D, OF_WBX : OF_WBX + L * RNB]
            wbr_sb = cstF[0:D, OF_WBR : OF_WBR + L * RNB]
            bias_sb = cstF[0:HID, OF_BIAS : OF_BIAS + L]
            bh_sb = cstF[0:HID, OF_BH : OF_BH + L]
            relT_sb = cstF[0:D, OF_RELT : OF_RELT + R]
            mask_sb = cstF[0:128, OF_MASK : OF_MASK + CH]
            ones = cstF[0:1, OF_ONES : OF_ONES + 128]
            ones128 = cstF[0:128, OF_ONES128 : OF_ONES128 + 1]
            identF = cstF[0:128, OF_IDF : OF_IDF + 128]
            ident_sb = cstH[0:128, OF_IDH : OF_IDH + 128]
            ww_sb = cstH[0:NB, OF_WW : OF_WW + L * HID]
            wh_sb = cstH[0:HID, OF_WH : OF_WH + L * HID]

            # ---- resident adjT relations: (p, (c t i)) fp8 ----
            res_tiles = []
            for r in range(R):
                rt = resp.tile([128, C2 * 2 * E2], fp8, tag=f"res{r}", name=f"res{r}")
                nc.sync.dma_start(
                    out=rt[:, :].rearrange("p (c t i) -> p c t i", c=C2, t=2),
                    in_=adjT[r].rearrange("c p t i -> p c t i"),
                )
                res_tiles.append(rt)
            res_views = [
                res_tiles[r][:, :].rearrange("p (c t i) -> p c t i", c=C2, t=2)
                for r in range(R)
            ]

            # ---- layers ----
            for i in range(L):
                wbx_i = cstF[0:D, OF_WBX + i * RNB : OF_WBX + (i + 1) * RNB]
                wbr_i = cstF[0:D, OF_WBR + i * RNB : OF_WBR + (i + 1) * RNB]
                ww_i = cstH[0:NB, OF_WW + i * HID : OF_WW + (i + 1) * HID]
                wh_i = cstH[0:HID, OF_WH + i * HID : OF_WH + (i + 1) * HID]
                bias_i = cstF[0:HID, OF_BIAS + i : OF_BIAS + i + 1]
                bh_i = cstF[0:HID, OF_BH + i : OF_BH + i + 1]

                # c[r, :] = rel_r @ Wb_rel[i, r] -> psy partition 0, tail cols
                psy = psY.tile([128, CH * RNB + YQ], f32, tag="y", name=f"psy{i}")
                for r in range(R):
                    nc.tensor.matmul(
                        psy[0:1, CH * RNB + 3 * r : CH * RNB + 3 * r + 3],
                        cstF[0:D, OF_RELT + r : OF_RELT + r + 1],
                        wbr_i[:, 3 * r : 3 * r + 3],
                        start=True, stop=True,
                    )
                c_sb = workp.tile([1, RNB], f32, tag="c_sb", name=f"c_sb{i}", bufs=2)
                nc.scalar.copy(out=c_sb[:, :], in_=psy[0:1, CH * RNB : CH * RNB + RNB])

                # y[kchunk] = x[kchunk] @ Wbx[i] + 1 (x) c  -> fp8 (128, 30)/chunk
                for k in range(CH):
                    ks = slice(k * RNB, (k + 1) * RNB)
                    nc.tensor.matmul(
                        psy[:, ks], xT[:, k * 128 : (k + 1) * 128], wbx_i,
                        start=True, stop=False,
                    )
                    nc.tensor.matmul(
                        psy[:, ks], cstF[0:1, OF_ONES : OF_ONES + 128], c_sb[:, :],
                        start=False, stop=True,
                    )
                y_all = ypool.tile([128, CH * YQ], fp8, tag="y_all", name=f"y_all{i}")
                nc.scalar.copy(
                    out=y_all[:, :].rearrange("p (k q) -> p k q", k=CH)[:, :, 0:RNB],
                    in_=psy[:, 0 : CH * RNB].rearrange("p (k q) -> p k q", k=CH),
                )
                y_view = y_all[:, :].rearrange("p (k q) -> p k q", q=YQ)

                # Z[ic] (128 i, 3) += adjT_block.T @ y_chunk   (DoubleRow fp8)
                zps = psZ.tile([128, IC * 8], f32, tag="z", name=f"zps{i}")
                for ic in range(IC):
                    i0, iw = ISL[ic]
                    for r in range(R):
                        for c in range(C2):
                            nc.tensor.matmul(
                                zps[0:iw, ic * 8 : ic * 8 + NB],
                                res_views[r][:, c, :, i0 : i0 + iw],
                                y_view[:, 2 * c : 2 * c + 2, 3 * r : 3 * r + 3],
                                start=(r == 0 and c == 0),
                                stop=(r == R - 1 and c == C2 - 1),
                                perf_mode=DR,
                            )
                zc_sb = workp.tile([128, IC * NB], bf16, tag="zc",
                                   name=f"zc{i}", bufs=2)
                nc.scalar.copy(
                    out=zc_sb[:, :].rearrange("p (k w) -> p k w", w=NB),
                    in_=zps[:, :].rearrange("p (k w) -> p k w", w=8)[:, :, 0:NB])

                # transpose Z chunks -> zT (3, E2) bf16
                zt_ps = [
                    psT.tile([NB, 512], bf16, tag="zt0", name=f"zt0_{i}", bufs=1),
                    psT.tile([NB, 512], bf16, tag="zt1", name=f"zt1_{i}", bufs=1),
                    psT.tile([NB, 512], bf16, tag="zt2", name=f"zt2_{i}", bufs=1),
                ]
                for ic in range(IC):
                    i0, iw = ISL[ic]
                    pst = zt_ps[ic // 4]
                    off = (ic % 4) * 128
                    nc.tensor.transpose(
                        pst[:, off : off + iw],
                        zc_sb[0:iw, ic * NB : (ic + 1) * NB],
                        ident_sb[0:iw, 0:iw],
                    )
                zT_chunks = []
                for n in range(3):
                    n0, nw = NSL[n]
                    ztc = workp.tile([NB, 512], bf16, tag="zT", name=f"zT{i}_{n}",
                                     bufs=2)
                    nc.vector.tensor_copy(ztc[:, 0:nw], zt_ps[n][:, 0:nw])
                    zT_chunks.append(ztc)

                # tail: h = relu(Z @ Ww + bias); g = sigmoid(h @ Wh + bh);
                # x += g * (h - x)
                for n in range(3):
                    n0, nw = NSL[n]
                    ns = slice(n0, n0 + nw)
                    psh = psB.tile([HID, 512], f32, tag="hh", bufs=1,
                                   name=f"psh{i}_{n0}")
                    nc.tensor.matmul(
                        psh[:, 0:nw], ww_i, zT_chunks[n][:, 0:nw],
                        start=True, stop=True,
                    )
                    hc = workp.tile([HID, 512], bf16, tag="h", name=f"h{i}_{n}",
                                    bufs=2)
                    nc.scalar.activation(
                        hc[:, 0:nw], psh[:, 0:nw], AF.Relu, bias=bias_i,
                    )
                    psg = psB.tile([HID, 512], f32, tag="gg", bufs=2,
                                   name=f"psg{i}_{n0}")
                    nc.tensor.matmul(
                        psg[:, 0:nw], wh_i, hc[:, 0:nw],
                        start=True, stop=True,
                    )
                    gc = workp.tile([HID, 512], bf16, tag="g", name=f"g{i}_{n}",
                                    bufs=2)
                    nc.scalar.activation(
                        gc[:, 0:nw], psg[:, 0:nw], AF.Sigmoid, bias=bh_i,
                    )
                    nc.vector.tensor_sub(hc[:, 0:nw], hc[:, 0:nw], xT[:, ns])
                    nc.vector.tensor_mul(hc[:, 0:nw], hc[:, 0:nw], gc[:, 0:nw])
                    nc.vector.tensor_add(xT[:, ns], xT[:, ns], hc[:, 0:nw])

            # ---- masked mean over entities ----
            # gsum[d] = sum_k (xT chunk k).T[e, d] * maskP[e, k] via PE:
            # transpose each (100,128) x chunk to (128,100), then accumulate
            # matmul with the mask column as stationary.
            gsum_ps = psB.tile([HID, 512], f32, tag="gg", bufs=2, name="gsum_ps")
            for k in range(CH):
                xt_ps = psY.tile([128, CH * RNB + YQ], f32, tag="y",
                                 name=f"xtp{k}")
                nc.tensor.transpose(
                    xt_ps[:, 0:HID],
                    xT[:, k * 128 : (k + 1) * 128],
                    cstF[0:HID, OF_IDF : OF_IDF + HID],
                )
                x_im = workp.tile([128, HID], f32, tag="x_im",
                                  name=f"x_im{k}", bufs=2)
                nc.scalar.copy(out=x_im[:, :], in_=xt_ps[:, 0:HID])
                nc.tensor.matmul(
                    gsum_ps[:, 0:1], x_im[:, :], cstF[0:128, OF_MASK + k : OF_MASK + k + 1],
                    start=(k == 0), stop=(k == CH - 1),
                )
            # den = max(sum(mask), 1); gsum /= den
            mrow = workp.tile([128, 1], f32, tag="mrow", name="mrow", bufs=1)
            nc.vector.reduce_sum(mrow[:, :], mask_sb, axis=AX.X)
            den_ps = psB.tile([HID, 512], f32, tag="hh", bufs=1, name="den_ps")
            nc.tensor.matmul(
                den_ps[0:1, 0:1], ones128, mrow[:, :], start=True, stop=True,
            )
            den1 = workp.tile([1, 1], f32, tag="den", name="den", bufs=1)
            nc.vector.tensor_scalar_max(den1[:, :], den_ps[0:1, 0:1], 1.0)
            nc.vector.reciprocal(den1[:, :], den1[:, :])
            psd = psB.tile([HID, 512], f32, tag="hh", bufs=1, name="psd")
            nc.tensor.matmul(
                psd[:, 0:1], cstF[0:1, OF_ONES : OF_ONES + HID], den1[:, :], start=True, stop=True,
            )
            rden = workp.tile([HID, 1], f32, tag="rden", name="rden", bufs=1)
            nc.scalar.copy(out=rden[:, :], in_=psd[:, 0:1])
            gsum = workp.tile([HID, 1], f32, tag="gsum", name="gsum", bufs=1)
            nc.vector.tensor_mul(gsum[:, :], gsum_ps[:, 0:1], rden[:, :])
            nc.sync.dma_start(out=graphD, in_=gsum[:, :])

    nc.compile()
    return nc


def get_nc():
    if "nc" not in _NC_CACHE:
        _NC_CACHE["nc"] = _build_nc()
    return _NC_CACHE["nc"]


def make_in_maps(adj, mask_ids, ent_emb, rel_emb, Wb, Ww, bias, Wh, bh):
    adj = np.asarray(adj, dtype=np.float32)
    pad = np.zeros((B, R, EP, E2), dtype=ml_dtypes.float8_e4m3fn)
    pad[:, :, :E, :E] = adj.transpose(0, 1, 3, 2).astype(ml_dtypes.float8_e4m3fn)
    # [b, r, c, p, t, i] = adj[b, r, i, j = c*256 + t*128 + p]
    adjT = np.ascontiguousarray(
        pad.reshape(B, R, C2, 2, 128, E2).transpose(0, 1, 2, 4, 3, 5)
    )
    entT = np.zeros((D, EP), dtype=np.float32)
    entT[:, :E] = np.asarray(ent_emb, np.float32).T
    relTh = np.ascontiguousarray(np.asarray(rel_emb, np.float32).T)
    Wb5 = np.asarray(Wb, np.float32).reshape(L, R, 2, D, NB)
    wbx = np.ascontiguousarray(
        Wb5[:, :, 0].transpose(0, 2, 1, 3).reshape(L, D, RNB)
    )
    wbr = np.ascontiguousarray(
        Wb5[:, :, 1].transpose(0, 2, 1, 3).reshape(L, D, RNB)
    )
    maskf = np.asarray(mask_ids).astype(np.float32)
    cstF_ = np.zeros((128, CBF), np.float32)
    cstF_[0:128, OF_IDF:OF_IDF+128] = np.eye(128, dtype=np.float32)
    cstF_[0:D, OF_WBX:OF_WBX+L*RNB] = wbx.transpose(1, 0, 2).reshape(D, L*RNB)
    cstF_[0:D, OF_WBR:OF_WBR+L*RNB] = wbr.transpose(1, 0, 2).reshape(D, L*RNB)
    cstF_[0:HID, OF_BIAS:OF_BIAS+L] = np.asarray(bias, np.float32).T
    cstF_[0:HID, OF_BH:OF_BH+L] = np.asarray(bh, np.float32).T
    cstF_[0:D, OF_RELT:OF_RELT+R] = relTh
    cstF_[0:1, OF_ONES:OF_ONES+128] = 1.0
    cstF_[0:128, OF_ONES128] = 1.0
    cstH_ = np.zeros((128, CBH), ml_dtypes.bfloat16)
    cstH_[0:128, OF_IDH:OF_IDH+128] = np.eye(128, dtype=np.float32)
    wwf = np.asarray(Ww, np.float32)   # (L, NB, HID)
    cstH_[0:NB, OF_WW:OF_WW+L*HID] = wwf.transpose(1, 0, 2).reshape(NB, L*HID)
    whf = np.asarray(Wh, np.float32)   # (L, HID, HID)
    cstH_[0:HID, OF_WH:OF_WH+L*HID] = whf.transpose(1, 0, 2).reshape(HID, L*HID)
    common = dict(xT0=entT, cstF=cstF_, cstH=np.ascontiguousarray(cstH_))
    in_maps = []
    for c in range(8):
        b = c // 2
        m = dict(common)
        m["adjT"] = np.ascontiguousarray(adjT[b])
        mp = np.zeros((EP,), dtype=np.float32)
        mp[:E] = maskf[b]
        cf = common["cstF"].copy()
        cf[0:128, OF_MASK:OF_MASK+CH] = mp.reshape(CH, 128).T
        m["cstF"] = cf
        in_maps.append(m)
    return in_maps


def run(inputs, trace=False):
    nc = get_nc()
    in_maps = make_in_maps(**{k: np.asarray(v) for k, v in inputs.items()})
    res = bass_utils.run_bass_kernel_spmd(
        nc, in_maps, core_ids=list(range(8)), trace=trace
    )
    out = np.stack(
        [np.asarray(res.results[2 * b]["graph"]).reshape(HID) for b in range(B)]
    ).astype(np.float32)
    return out, res


def kernel(**inputs):
    out, _ = run(inputs, trace=False)
    return out
